# revision 3
# baseline (speedup 1.0000x reference)
"""AttentionReadout Trainium2 kernel (8-core data-parallel over graphs).

Reference computation (per graph of 64 nodes, D=512, H=8 heads, hd=64):
    qkv = x @ in_proj_w.T + in_proj_b ; q,k,v = split(qkv)
    attn = softmax(q k^T / sqrt(hd)) v          (per head)
    attn_out = attn @ out_proj_w.T + out_proj_b
    gates = sigmoid(attn_out @ gate_w.T + gate_b)
    out[g] = sum_n attn_out[n] * gates[n]

Key algebraic restructure vs the naive chain: with weff = out_proj_w.T @
gate_w and gb_eff = gate_b + out_proj_b @ gate_w,
    gates  = sigmoid(ctx @ weff + gb_eff)           (no attn_out needed)
    out[g] = (sum_n gates_n * ctx_n) @ out_proj_w.T + (sum_n gates_n) * bo
so the out-projection runs ONCE per core on [128 graphs, D], not per node.

Layout strategy (per core: 128 graphs = 8192 nodes, superblock = 512 nodes):
  - x arrives PRE-TRANSPOSED from the host ([128, DC, rows] bf16): no
    on-device transposes, plain contiguous DMA loads only.
  - Q^T,K^T projected in [e, n] orientation one superblock AHEAD. Odd
    heads are consumed straight from partitions 64:128 via PE row-tiling
    (tile_position (64,0)) -- no realignment bounce; even/odd head score
    matmuls run concurrently in disjoint row groups.
  - scores for all 8 heads of a 128-node block go to one [128, 8, 128]
    psum (2 banks: even-head slots 0-3 bank A, odd slots 4-7 bank B so
    concurrent drains hit different banks). exp runs as TWO 512-elem
    ScalarE instructions per block (diag quadrants only; attn buffers
    pre-zeroed off-diagonal).
  - ctx natural [n, e] per head via stationary attn / moving
    [v | vg | ones] (N=66: ctx + gate numerator + rowsum in one shot)
    into one [128, 8, 128] psum; ONE reciprocal + ONE normalize + ONE
    zp instruction per block.
  - gate: zp accumulated per superblock, ONE tanh [128, 4] per sb,
    G written via 8 tiny gpsimd ops into per-block zeroed G tiles.
  - readout: per block one matmul, stationary G[128, 32], moving ctxn
    [128, 512], accumulated per 4-superblock group into an exclusive
    psum bank (col tile_position 32k) -> r[g, e] for all 128 graphs.
  - tail: r -> (PE transpose) -> r^T -> 4 matmuls vs wo -> out.
  - ~3.5us of tiny dummy matmuls at t=0 warm the PE HAM clock gate so
    real matmuls start at 2.4 GHz, hidden under the initial weight DMA.
"""

import numpy as np
import ml_dtypes

import concourse.bass as bass
import concourse.mybir as mybir
import concourse.tile as tile
from concourse import bacc
from concourse.bass_utils import run_bass_kernel_spmd
from concourse.masks import make_identity

F32 = mybir.dt.float32
BF16 = mybir.dt.bfloat16

N_CORES = 8
D = 512
H = 8
HD = 64
NPG = 64            # nodes per graph
TOTAL = 65536
ROWS = TOTAL // N_CORES      # 8192 nodes per core
GC = ROWS // NPG             # 128 graphs per core
BLK = 128                    # nodes per block (2 graphs)
SBN = 512                    # nodes per superblock (4 blocks, 8 graphs)
NSB = ROWS // SBN            # 16 superblocks
NBLK = SBN // BLK            # 4 blocks per superblock
DC = D // 128                # 4 d-chunks
GRP = 4                      # superblocks per readout group (32 graphs)
NDUM = 60                    # HAM warm-up dummy matmuls

# module-level switch used by test.py; harness default is no tracing
TRACE = False

try:
    import jax as _jax
    _jax.config.update("jax_compilation_cache_dir", "/tmp/jax_neff_cache")
    _jax.config.update("jax_persistent_cache_min_compile_time_secs", 10)
    _jax.config.update("jax_persistent_cache_min_entry_size_bytes", 0)
except Exception:
    pass


def _build(has_bqk, has_bv, has_bo, has_gb, gb_eff=0.0, rows=ROWS):
    nsb = rows // SBN
    gc = rows // NPG
    nc = bacc.Bacc(None, target_bir_lowering=False, debug=False)

    xbf = nc.dram_tensor("xbf", [128, DC, rows], BF16, kind="ExternalInput")
    wqk = nc.dram_tensor("wqk", [128, DC, 2 * D], BF16, kind="ExternalInput")
    wv = nc.dram_tensor("wv", [128, DC, D], BF16, kind="ExternalInput")
    wo = nc.dram_tensor("wo", [128, DC, D], BF16, kind="ExternalInput")
    weff = nc.dram_tensor("weff", [1, D], F32, kind="ExternalInput")
    if has_bqk:
        bqk = nc.dram_tensor("bqk", [128, 8], F32, kind="ExternalInput")
    if has_bv:
        bv = nc.dram_tensor("bv", [1, D], F32, kind="ExternalInput")
    if has_bo:
        bo = nc.dram_tensor("bo", [1, D], F32, kind="ExternalInput")
    out = nc.dram_tensor("out", [gc, D], F32, kind="ExternalOutput")

    from contextlib import ExitStack
    with tile.TileContext(nc) as tc, ExitStack() as st:
        consts = st.enter_context(tc.tile_pool(name="consts", bufs=1))
        p_xt = st.enter_context(tc.tile_pool(name="p_xt", bufs=3))
        p_qkt = st.enter_context(tc.tile_pool(name="p_qkt", bufs=2))
        p_vtx = st.enter_context(tc.tile_pool(name="p_vtx", bufs=3))
        p_attn = st.enter_context(tc.tile_pool(name="p_attn", bufs=2))
        p_ctxn = st.enter_context(tc.tile_pool(name="p_ctxn", bufs=6))
        p_small = st.enter_context(tc.tile_pool(name="p_small", bufs=3))
        p_G = st.enter_context(tc.tile_pool(name="p_G", bufs=16))
        p_out = st.enter_context(tc.tile_pool(name="p_out", bufs=1))
        ps_big = st.enter_context(tc.tile_pool(name="ps_big", bufs=2, space="PSUM"))
        ps_s = st.enter_context(tc.tile_pool(name="ps_s", bufs=1, space="PSUM"))
        ps_c = st.enter_context(tc.tile_pool(name="ps_c", bufs=1, space="PSUM"))
        ps_r = st.enter_context(tc.tile_pool(name="ps_r", bufs=1, space="PSUM"))
        ps_t = st.enter_context(tc.tile_pool(name="ps_t", bufs=1, space="PSUM"))

        # ---- HAM warm-up: tiny matmuls on zeroed SBUF, one accumulation
        # group into the tail psum bank, issued before any DMA-dependent
        # work so the PE clock gate is at 8/8 when real matmuls start
        zstat = consts.tile([64, 1], BF16, tag="zstat")
        nc.gpsimd.memset(zstat[:], 0.0)
        zmov = consts.tile([64, 64], BF16, tag="zmov")
        nc.gpsimd.memset(zmov[:], 0.0)
        dumps = ps_t.tile([128, DC, 128], F32, tag="tail", name="dumps")
        for i in range(NDUM):
            nc.tensor.matmul(
                dumps[0:1, 0, 0:64], zstat[:], zmov[:],
                start=(i == 0), stop=(i == NDUM - 1))

        # ---- weights / constants; order matters for startup overlap ----
        xt0 = p_xt.tile([128, DC, SBN], BF16, tag="xt", name="xt0")
        nc.sync.dma_start(xt0[:, :, :], xbf[:, :, 0:SBN])
        wqk_sb = consts.tile([128, DC, 2 * D], BF16, tag="wqk")
        nc.sync.dma_start(wqk_sb[:, :, 0:D // 2], wqk[:, :, 0:D // 2])
        wv_sb = consts.tile([128, DC, D], BF16, tag="wv")
        nc.sync.dma_start(wv_sb[:], wv[:, :, :])
        for q in range(1, 4):
            nc.sync.dma_start(
                wqk_sb[:, :, q * D // 2:(q + 1) * D // 2],
                wqk[:, :, q * D // 2:(q + 1) * D // 2])

        weff_row = consts.tile([1, D], F32, tag="weff_row")
        nc.sync.dma_start(weff_row[:], weff[:, :])
        weff_f32 = consts.tile([128, D], F32, tag="weff_f32")
        nc.gpsimd.partition_broadcast(weff_f32[:], weff_row[:])
        weff_bc = consts.tile([128, D], BF16, tag="weff_bc")
        nc.vector.tensor_copy(weff_bc[:], weff_f32[:])

        if has_bqk:
            bqk_sb = consts.tile([128, 8], F32, tag="bqk")
            nc.sync.dma_start(bqk_sb[:], bqk[:, :])
        if has_bv:
            bv_row = consts.tile([1, D], F32, tag="bv_row")
            nc.sync.dma_start(bv_row[:], bv[:, :])
            bv_full = consts.tile([128, D], F32, tag="bv_full")
            nc.gpsimd.partition_broadcast(bv_full[:], bv_row[:])
        if has_bo:
            bo_row = consts.tile([1, D], F32, tag="bo_row")
            nc.sync.dma_start(bo_row[:], bo[:, :])
            bo_full = consts.tile([128, D], F32, tag="bo_full")
            nc.gpsimd.partition_broadcast(bo_full[:], bo_row[:])
            s_row = consts.tile([1, gc], F32, tag="s_row")

        # readout accumulator: one psum bank, exclusively owned
        rps = ps_r.tile([128, D], F32, tag="r")

        # pre-zero attn pool buffers' off-diagonal quadrants (they are
        # never dirtied: exp writes only the diagonal quadrants)
        for _ in range(2):
            az = p_attn.tile([128, H, BLK], BF16, tag="attn", name="az")
            nc.gpsimd.memset(az[0:64, :, 64:128], 0.0)
            nc.gpsimd.memset(az[64:128, :, 0:64], 0.0)
        # pre-set vtx ones column (layout: [v 0:64 | vg 64 | ones 65])
        for _ in range(3):
            vz = p_vtx.tile([128, H, HD + 2], BF16, tag="vtx", name="vz")
            nc.vector.memset(vz[:, :, HD + 1:HD + 2], 1.0)
        # pre-zero G buffers
        for _ in range(16):
            gz_ = p_G.tile([128, 32], BF16, tag="G", name="gzb")
            nc.gpsimd.memset(gz_[:], 0.0)

        # tail-only constants, emitted last so they never gate the loop
        ident_f32 = consts.tile([128, 128], F32, tag="ident_f32")
        make_identity(nc, ident_f32[:])
        wo_sb = consts.tile([128, DC, D], BF16, tag="wo")
        nc.sync.dma_start(wo_sb[:], wo[:, :, :])

        # ---- helpers ----
        pending_ro = []

        def emit_xt(s):
            t = p_xt.tile([128, DC, SBN], BF16, tag="xt", name="xt")
            nc.sync.dma_start(t[:, :, :], xbf[:, :, s * SBN:(s + 1) * SBN])
            return t

        def emit_qk_ec(xt_n, qkt_n, ec):
            ps = ps_big.tile([128, SBN], F32, tag="big", name="psq")
            for dc in range(DC):
                nc.tensor.matmul(
                    ps[:],
                    wqk_sb[:, dc, ec * 128:(ec + 1) * 128],
                    xt_n[:, dc, :],
                    start=(dc == 0), stop=(dc == DC - 1))
            if has_bqk:
                if ec % 2 == 0:
                    nc.vector.tensor_scalar_add(
                        qkt_n[:, ec, :], ps[:], bqk_sb[:, ec:ec + 1])
                else:
                    nc.scalar.activation(
                        qkt_n[:, ec, :], ps[:],
                        mybir.ActivationFunctionType.Identity,
                        bias=bqk_sb[:, ec:ec + 1])
            else:
                if ec % 2 == 0:
                    nc.vector.tensor_copy(qkt_n[:, ec, :], ps[:])
                else:
                    nc.scalar.copy(qkt_n[:, ec, :], ps[:])

        def emit_v(xt, b, vts):
            psv = ps_big.tile([128, SBN], F32, tag="big", name="psv")
            for dc in range(DC):
                nc.tensor.matmul(
                    psv[:],
                    xt[:, dc, b * 128:(b + 1) * 128],
                    wv_sb[:, dc, :],
                    start=(dc == 0), stop=(dc == DC - 1))
            vtx = p_vtx.tile([128, H, HD + 2], BF16, tag="vtx")
            pv = psv[:].rearrange("p (h c) -> p h c", h=H)
            if has_bv:
                nc.vector.tensor_tensor(
                    vtx[:, :, 0:HD], pv,
                    bv_full[:].rearrange("p (h c) -> p h c", h=H),
                    mybir.AluOpType.add)
            else:
                nc.vector.tensor_copy(vtx[:, :, 0:HD], pv)
            # vg[n, h] = v[n, h, :] . weff[h, :] (gate numerator seed):
            # multiply on gpsimd (has slack), reduce on DVE, scatter gpsimd
            scr = p_small.tile([128, H, HD], BF16, tag="scr")
            nc.gpsimd.tensor_tensor(
                scr[:], vtx[:, :, 0:HD],
                weff_bc[:].rearrange("p (h c) -> p h c", h=H),
                mybir.AluOpType.mult)
            vgt = p_small.tile([128, H], F32, tag="vgt")
            nc.vector.tensor_reduce(
                vgt[:], scr[:], mybir.AxisListType.X, mybir.AluOpType.add)
            nc.gpsimd.tensor_copy(vtx[:, :, HD], vgt[:])
            vts[b] = vtx

        def emit_scores(qkt, b, attns):
            n0 = b * BLK
            S = ps_s.tile([128, H, BLK], F32, tag="s")
            for c in range(4):
                # even head 2c -> slot c (bank A), odd 2c+1 -> slot 4+c
                # (bank B); row-tiled pairs run concurrently
                nc.tensor.matmul(
                    S[:, c, :],
                    qkt[0:64, 4 + c, n0:n0 + BLK],
                    qkt[0:64, c, n0:n0 + BLK],
                    start=True, stop=True)
                nc.tensor.matmul(
                    S[:, 4 + c, :],
                    qkt[64:128, 4 + c, n0:n0 + BLK],
                    qkt[64:128, c, n0:n0 + BLK],
                    start=True, stop=True)
            attn = p_attn.tile([128, H, BLK], BF16, tag="attn")
            nc.scalar.activation(
                attn[0:64, :, 0:64], S[0:64, :, 0:64],
                mybir.ActivationFunctionType.Exp, scale=0.125)
            nc.scalar.activation(
                attn[64:128, :, 64:128], S[64:128, :, 64:128],
                mybir.ActivationFunctionType.Exp, scale=0.125)
            attns[b] = attn

        def emit_ctx(b, attn, vtx, zp_sb, ctxns):
            psc = ps_c.tile([128, H, BLK], F32, tag="c")
            for h in range(H):
                s = (h // 2) + 4 * (h % 2)
                nc.tensor.matmul(
                    psc[:, h, 0:HD + 2],
                    attn[:, s, :],
                    vtx[:, h, :],
                    start=True, stop=True)
            rr = p_small.tile([128, H], F32, tag="rr")
            nc.vector.reciprocal(rr[:], psc[:, :, HD + 1])
            ctxn = p_ctxn.tile([128, H, HD], BF16, tag="ctxn")
            nc.vector.tensor_tensor(
                ctxn[:],
                psc[:, :, 0:HD],
                rr[:, :, None].to_broadcast((128, H, HD)),
                mybir.AluOpType.mult)
            nc.vector.tensor_tensor(
                zp_sb[:, b, :], psc[:, :, HD], rr[:],
                mybir.AluOpType.mult)
            ctxns[b] = ctxn

        def emit_gate_and_ro(sb, zp_sb, ctxns):
            gzs = p_small.tile([128, NBLK], F32, tag="gzs")
            nc.vector.tensor_reduce(
                gzs[:], zp_sb[:], mybir.AxisListType.X, mybir.AluOpType.add)
            gt = p_small.tile([128, NBLK], F32, tag="gt")
            nc.scalar.activation(
                gt[:], gzs[:], mybir.ActivationFunctionType.Tanh,
                bias=(0.5 * gb_eff) if has_gb else 0.0, scale=0.5)
            k = sb // GRP
            for b in range(NBLK):
                G = p_G.tile([128, 32], BF16, tag="G")
                c0 = 8 * (sb % GRP) + 2 * b
                nc.gpsimd.tensor_scalar(
                    G[0:64, c0:c0 + 1], gt[0:64, b:b + 1], 0.5, 0.5,
                    mybir.AluOpType.mult, mybir.AluOpType.add)
                nc.gpsimd.tensor_scalar(
                    G[64:128, c0 + 1:c0 + 2], gt[64:128, b:b + 1], 0.5, 0.5,
                    mybir.AluOpType.mult, mybir.AluOpType.add)
                if has_bo:
                    g0 = 8 * sb + 2 * b
                    nc.gpsimd.tensor_reduce(
                        s_row[0:1, g0:g0 + 1], gt[0:64, b:b + 1],
                        mybir.AxisListType.C, mybir.AluOpType.add)
                    nc.gpsimd.tensor_reduce(
                        s_row[0:1, g0 + 1:g0 + 2], gt[64:128, b:b + 1],
                        mybir.AxisListType.C, mybir.AluOpType.add)
                first = (sb % GRP == 0) and (b == 0)
                last = (sb % GRP == GRP - 1) and (b == NBLK - 1)
                cflat = ctxns[b][:].rearrange("p h c -> p (h c)")

                def _ro(G=G, cflat=cflat, k=k, first=first, last=last):
                    nc.tensor.matmul(
                        rps[32 * k:32 * k + 32, :], G[:], cflat,
                        start=first, stop=last,
                        tile_position=(0, 32 * k))
                pending_ro.append(_ro)

        # ---- prologue: QK for superblock 0 ----
        xts = {0: xt0}
        if nsb > 1:
            xts[1] = emit_xt(1)
        qkt0 = p_qkt.tile([128, H, SBN], BF16, tag="qkt", name="qkt0")
        for ec in range(8):
            emit_qk_ec(xts[0], qkt0, ec)
        qks = {0: qkt0}

        # ---- main loop ----
        for sb in range(nsb):
            if sb + 2 < nsb:
                xts[sb + 2] = emit_xt(sb + 2)
            xt = xts.pop(sb)
            qkt = qks.pop(sb)

            # QK projection groups for the NEXT superblock, interleaved
            # into this superblock's work
            if sb + 1 < nsb:
                xt_n = xts[sb + 1]
                qkt_n = p_qkt.tile([128, H, SBN], BF16, tag="qkt",
                                   name="qktn")
                qgrps = [lambda ec=ec: emit_qk_ec(xt_n, qkt_n, ec)
                         for ec in range(8)]
                qks[sb + 1] = qkt_n
            else:
                qgrps = []

            vts = [None] * NBLK
            attns = [None] * NBLK
            ctxns = [None] * NBLK
            zp_sb = p_small.tile([128, NBLK, H], F32, tag="zp_sb")
            ros = pending_ro
            pending_ro = []

            def q1(n=1):
                for _ in range(n):
                    if qgrps:
                        qgrps.pop(0)()

            def ro1(n=2):
                for _ in range(n):
                    if ros:
                        ros.pop(0)()

            # PE emission order: V groups early (vg chain latency), QK
            # groups fill exp/norm dependency gaps, readouts of the
            # previous superblock sprinkled in
            emit_v(xt, 0, vts)
            emit_scores(qkt, 0, attns)
            ro1(2)
            q1()
            emit_v(xt, 1, vts)
            ro1(2)
            q1()
            emit_scores(qkt, 1, attns)
            emit_ctx(0, attns[0], vts[0], zp_sb, ctxns)
            emit_v(xt, 2, vts)
            q1()
            emit_scores(qkt, 2, attns)
            emit_ctx(1, attns[1], vts[1], zp_sb, ctxns)
            emit_v(xt, 3, vts)
            q1()
            emit_scores(qkt, 3, attns)
            emit_ctx(2, attns[2], vts[2], zp_sb, ctxns)
            q1(2)
            emit_ctx(3, attns[3], vts[3], zp_sb, ctxns)
            q1(2)
            emit_gate_and_ro(sb, zp_sb, ctxns)

        # ---- tail: r -> r^T -> out projection ----
        while pending_ro:
            pending_ro.pop(0)()
        rsb = p_out.tile([128, D], F32, tag="rsb")
        nc.vector.tensor_copy(rsb[:], rps[:])
        ptt = ps_t.tile([128, DC, 128], F32, tag="tail", name="ptt")
        for dc in range(DC):
            nc.tensor.transpose(
                ptt[:, dc, :], rsb[:, dc * 128:(dc + 1) * 128], ident_f32[:])
        rt = p_out.tile([128, DC, 128], BF16, tag="rt")
        nc.scalar.copy(rt[:], ptt[:])
        pso = ps_t.tile([128, D], F32, tag="tail", name="pso")
        for dc in range(DC):
            nc.tensor.matmul(
                pso[:], rt[:, dc, :], wo_sb[:, dc, :],
                start=(dc == 0), stop=(dc == DC - 1))
        out_sb = p_out.tile([128, D], F32, tag="osb")
        if has_bo:
            # out += (sum_n gate_n) * bo : transpose s_row to [gc, 1]
            pst = ps_c.tile([128, H, BLK], F32, tag="c", name="pst")
            nc.tensor.transpose(pst[0:gc, 0, 0:1], s_row[:, :], ident_f32[:])
            s_col = p_out.tile([128, 1], F32, tag="s_col")
            nc.vector.tensor_copy(s_col[0:gc, :], pst[0:gc, 0, 0:1])
            sbo = p_out.tile([128, D], F32, tag="sbo")
            nc.vector.tensor_scalar_mul(
                sbo[:], bo_full[:], s_col[:, 0:1])
            nc.vector.tensor_tensor(
                out_sb[:], pso[:], sbo[:], mybir.AluOpType.add)
        else:
            nc.vector.tensor_copy(out_sb[:], pso[:])
        nc.sync.dma_start(out[:, :], out_sb[0:gc, :])

    import time as _time
    _t = _time.time()
    nc.compile()
    print(f"[kernel] bacc compile: {_time.time()-_t:.1f}s", flush=True)
    return nc


def kernel(x, batch, in_proj_w, in_proj_b, out_proj_w, out_proj_b,
           gate_w, gate_b):
    x = np.ascontiguousarray(np.asarray(x, dtype=np.float32))
    in_proj_w = np.asarray(in_proj_w, dtype=np.float32)
    in_proj_b = np.asarray(in_proj_b, dtype=np.float32)
    out_proj_w = np.asarray(out_proj_w, dtype=np.float32)
    out_proj_b = np.asarray(out_proj_b, dtype=np.float32)
    gate_w = np.asarray(gate_w, dtype=np.float32)
    gate_b = np.asarray(gate_b, dtype=np.float32)

    # host-side weight prep
    wqkT = in_proj_w[:2 * D].T                              # [512, 1024]
    wqk_h = np.ascontiguousarray(
        wqkT.reshape(DC, 128, 2 * D).transpose(1, 0, 2)).astype(ml_dtypes.bfloat16)
    wvT = in_proj_w[2 * D:].T                               # [512, 512]
    wv_h = np.ascontiguousarray(
        wvT.reshape(DC, 128, D).transpose(1, 0, 2)).astype(ml_dtypes.bfloat16)
    woT = out_proj_w.T                                      # [512, 512]
    wo_h = np.ascontiguousarray(
        woT.reshape(DC, 128, D).transpose(1, 0, 2)).astype(ml_dtypes.bfloat16)
    weff_h = (out_proj_w.T @ gate_w[0]).astype(np.float32).reshape(1, D)

    bqk_np = in_proj_b[:2 * D]
    bv_np = in_proj_b[2 * D:]
    gb_eff = float(gate_b[0] + out_proj_b @ gate_w[0])
    has_bqk = bool(np.any(bqk_np))
    has_bv = bool(np.any(bv_np))
    has_bo = bool(np.any(out_proj_b))
    has_gb = gb_eff != 0.0

    import time as _time
    _t = _time.time()
    nc = _build(has_bqk, has_bv, has_bo, has_gb, gb_eff=gb_eff)
    print(f"[kernel] build total: {_time.time()-_t:.1f}s", flush=True)

    in_maps = []
    for c in range(N_CORES):
        xc = x[c * ROWS:(c + 1) * ROWS].astype(ml_dtypes.bfloat16)
        xct = np.ascontiguousarray(
            xc.T.reshape(DC, 128, ROWS).transpose(1, 0, 2))
        m = {
            "xbf": xct,
            "wqk": wqk_h, "wv": wv_h, "wo": wo_h, "weff": weff_h,
        }
        if has_bqk:
            m["bqk"] = np.ascontiguousarray(
                bqk_np.reshape(8, 128).T).astype(np.float32)
        if has_bv:
            m["bv"] = bv_np.reshape(1, D).astype(np.float32)
        if has_bo:
            m["bo"] = out_proj_b.reshape(1, D).astype(np.float32)
        in_maps.append(m)

    kernel.last_nc = nc
    kernel.last_in_maps = in_maps
    kernel.last_flags = (has_bqk, has_bv, has_bo, has_gb)

    res = run_bass_kernel_spmd(
        nc, in_maps, core_ids=list(range(N_CORES)), trace=TRACE)
    if TRACE:
        kernel.last_exec_time_ns = res.exec_time_ns
        kernel.last_results = res

    return np.concatenate([r["out"] for r in res.results], axis=0)


kernel.last_exec_time_ns = None
kernel.last_results = None
kernel.last_nc = None
kernel.last_in_maps = None


# revision 8
# speedup vs baseline: 1.0327x; 1.0327x over previous
"""AttentionReadout Trainium2 kernel (8-core data-parallel over graphs).

Reference computation (per graph of 64 nodes, D=512, H=8 heads, hd=64):
    qkv = x @ in_proj_w.T + in_proj_b ; q,k,v = split(qkv)
    attn = softmax(q k^T / sqrt(hd)) v          (per head)
    attn_out = attn @ out_proj_w.T + out_proj_b
    gates = sigmoid(attn_out @ gate_w.T + gate_b)
    out[g] = sum_n attn_out[n] * gates[n]

Key algebraic restructure vs the naive chain: with weff = out_proj_w.T @
gate_w and gb_eff = gate_b + out_proj_b @ gate_w,
    gates  = sigmoid(ctx @ weff + gb_eff)           (no attn_out needed)
    out[g] = (sum_n gates_n * ctx_n) @ out_proj_w.T + (sum_n gates_n) * bo
so the out-projection runs ONCE per core on [128 graphs, D], not per node.

Layout strategy (per core: 128 graphs = 8192 nodes, superblock = 512 nodes):
  - x arrives PRE-TRANSPOSED from the host ([128, DC, rows] bf16): no
    on-device transposes, plain contiguous DMA loads only.
  - Q^T,K^T projected in [e, n] orientation one superblock AHEAD. Odd
    heads are consumed straight from partitions 64:128 via PE row-tiling
    (tile_position (64,0)) -- no realignment bounce; even/odd head score
    matmuls run concurrently in disjoint row groups.
  - scores for all 8 heads of a 128-node block go to one [128, 8, 128]
    psum (2 banks: even-head slots 0-3 bank A, odd slots 4-7 bank B so
    concurrent drains hit different banks). exp runs as TWO 512-elem
    ScalarE instructions per block (diag quadrants only; attn buffers
    pre-zeroed off-diagonal).
  - ctx natural [n, e] per head via stationary attn / moving
    [v | vg | ones] (N=66: ctx + gate numerator + rowsum in one shot)
    into one [128, 8, 128] psum; ONE reciprocal + ONE normalize + ONE
    zp instruction per block.
  - gate: zp accumulated per superblock, ONE tanh [128, 4] per sb,
    G written via 8 tiny gpsimd ops into per-block zeroed G tiles.
  - readout: per block one matmul, stationary G[128, 32], moving ctxn
    [128, 512], accumulated per 4-superblock group into an exclusive
    psum bank (col tile_position 32k) -> r[g, e] for all 128 graphs.
  - tail: r -> (PE transpose) -> r^T -> 4 matmuls vs wo -> out.
  - ~3.5us of tiny dummy matmuls at t=0 warm the PE HAM clock gate so
    real matmuls start at 2.4 GHz, hidden under the initial weight DMA.
"""

import numpy as np
import ml_dtypes

import concourse.bass as bass
import concourse.mybir as mybir
import concourse.tile as tile
from concourse import bacc
from concourse.bass_utils import run_bass_kernel_spmd
from concourse.masks import make_identity

F32 = mybir.dt.float32
BF16 = mybir.dt.bfloat16

N_CORES = 8
D = 512
H = 8
HD = 64
NPG = 64            # nodes per graph
TOTAL = 65536
ROWS = TOTAL // N_CORES      # 8192 nodes per core
GC = ROWS // NPG             # 128 graphs per core
BLK = 128                    # nodes per block (2 graphs)
SBN = 512                    # nodes per superblock (4 blocks, 8 graphs)
NSB = ROWS // SBN            # 16 superblocks
NBLK = SBN // BLK            # 4 blocks per superblock
DC = D // 128                # 4 d-chunks
GRP = 4                      # superblocks per readout group (32 graphs)
NDUM = 60                    # HAM warm-up dummy matmuls

# module-level switch used by test.py; harness default is no tracing
TRACE = False

try:
    import jax as _jax
    _jax.config.update("jax_compilation_cache_dir", "/tmp/jax_neff_cache")
    _jax.config.update("jax_persistent_cache_min_compile_time_secs", 10)
    _jax.config.update("jax_persistent_cache_min_entry_size_bytes", 0)
except Exception:
    pass


def _build(has_bqk, has_bv, has_bo, has_gb, gb_eff=0.0, rows=ROWS):
    nsb = rows // SBN
    gc = rows // NPG
    nc = bacc.Bacc(None, target_bir_lowering=False, debug=False)

    xbf = nc.dram_tensor("xbf", [128, DC, rows], BF16, kind="ExternalInput")
    wqk = nc.dram_tensor("wqk", [128, DC, 2 * D], BF16, kind="ExternalInput")
    wv = nc.dram_tensor("wv", [128, DC, D], BF16, kind="ExternalInput")
    wo = nc.dram_tensor("wo", [128, DC, D], BF16, kind="ExternalInput")
    weff = nc.dram_tensor("weff", [1, D], F32, kind="ExternalInput")
    if has_bqk:
        bqk = nc.dram_tensor("bqk", [128, 8], F32, kind="ExternalInput")
    if has_bv:
        bv = nc.dram_tensor("bv", [1, D], F32, kind="ExternalInput")
    if has_bo:
        bo = nc.dram_tensor("bo", [1, D], F32, kind="ExternalInput")
    out = nc.dram_tensor("out", [gc, D], F32, kind="ExternalOutput")

    from contextlib import ExitStack
    with tile.TileContext(nc) as tc, ExitStack() as st:
        consts = st.enter_context(tc.tile_pool(name="consts", bufs=1))
        p_xt = st.enter_context(tc.tile_pool(name="p_xt", bufs=3))
        p_qkt = st.enter_context(tc.tile_pool(name="p_qkt", bufs=2))
        p_vtx = st.enter_context(tc.tile_pool(name="p_vtx", bufs=3))
        p_attn = st.enter_context(tc.tile_pool(name="p_attn", bufs=2))
        p_ctxn = st.enter_context(tc.tile_pool(name="p_ctxn", bufs=6))
        p_small = st.enter_context(tc.tile_pool(name="p_small", bufs=3))
        p_G = st.enter_context(tc.tile_pool(name="p_G", bufs=16))
        p_out = st.enter_context(tc.tile_pool(name="p_out", bufs=1))
        ps_big = st.enter_context(tc.tile_pool(name="ps_big", bufs=3, space="PSUM"))
        ps_s = st.enter_context(tc.tile_pool(name="ps_s", bufs=1, space="PSUM"))
        ps_c = st.enter_context(tc.tile_pool(name="ps_c", bufs=1, space="PSUM"))
        ps_r = st.enter_context(tc.tile_pool(name="ps_r", bufs=1, space="PSUM"))

        # ---- HAM warm-up: matmuls on zeroed SBUF, one accumulation
        # group into a big-pool psum bank, issued before any DMA-dependent
        # work so the PE clock gate is at 8/8 when real matmuls start.
        # Also preload the ACT spline table set so the first psum->sbuf
        # copies don't eat the ~2.7us ACT_TABLE_LOAD.
        zmov = consts.tile([64, 64], BF16, tag="zmov")
        nc.gpsimd.memset(zmov[:], 0.0)
        actwarm = consts.tile([1, 1], F32, tag="actwarm")
        nc.scalar.activation(
            actwarm[:], zmov[0:1, 0:1],
            mybir.ActivationFunctionType.Exp, scale=1.0)
        dumps = ps_big.tile([128, SBN], F32, tag="big", name="dumps")
        for i in range(NDUM):
            nc.tensor.matmul(
                dumps[0:64, 0:64], zmov[:], zmov[:],
                start=(i == 0), stop=(i == NDUM - 1))

        # ---- weights / constants; order matters for startup overlap ----
        xt0 = p_xt.tile([128, DC, SBN], BF16, tag="xt", name="xt0")
        nc.sync.dma_start(xt0[:, :, :], xbf[:, :, 0:SBN])
        wqk_sb = consts.tile([128, DC, 2 * D], BF16, tag="wqk")
        nc.sync.dma_start(wqk_sb[:, :, 0:D // 2], wqk[:, :, 0:D // 2])
        wv_sb = consts.tile([128, DC, D], BF16, tag="wv")
        nc.sync.dma_start(wv_sb[:], wv[:, :, :])
        for q in range(1, 4):
            nc.sync.dma_start(
                wqk_sb[:, :, q * D // 2:(q + 1) * D // 2],
                wqk[:, :, q * D // 2:(q + 1) * D // 2])

        weff_row = consts.tile([1, D], F32, tag="weff_row")
        nc.sync.dma_start(weff_row[:], weff[:, :])
        weff_f32 = consts.tile([128, D], F32, tag="weff_f32")
        nc.gpsimd.partition_broadcast(weff_f32[:], weff_row[:])
        weff_bc = consts.tile([128, D], BF16, tag="weff_bc")
        nc.vector.tensor_copy(weff_bc[:], weff_f32[:])

        if has_bqk:
            bqk_sb = consts.tile([128, 8], F32, tag="bqk")
            nc.sync.dma_start(bqk_sb[:], bqk[:, :])
        if has_bv:
            bv_row = consts.tile([1, D], F32, tag="bv_row")
            nc.sync.dma_start(bv_row[:], bv[:, :])
            bv_full = consts.tile([128, D], F32, tag="bv_full")
            nc.gpsimd.partition_broadcast(bv_full[:], bv_row[:])
        if has_bo:
            bo_row = consts.tile([1, D], F32, tag="bo_row")
            nc.sync.dma_start(bo_row[:], bo[:, :])
            bo_full = consts.tile([128, D], F32, tag="bo_full")
            nc.gpsimd.partition_broadcast(bo_full[:], bo_row[:])
            s_row = consts.tile([1, gc], F32, tag="s_row")

        # readout accumulator: one psum bank, exclusively owned
        rps = ps_r.tile([128, D], F32, tag="r")

        # pre-zero attn pool buffers' off-diagonal quadrants (they are
        # never dirtied: exp writes only the diagonal quadrants)
        for _ in range(2):
            az = p_attn.tile([128, H, BLK], BF16, tag="attn", name="az")
            nc.gpsimd.memset(az[0:64, :, 64:128], 0.0)
            nc.gpsimd.memset(az[64:128, :, 0:64], 0.0)
        # pre-set vtx ones column (layout: [v 0:64 | vg 64 | ones 65])
        for _ in range(3):
            vz = p_vtx.tile([128, H, HD + 2], BF16, tag="vtx", name="vz")
            nc.vector.memset(vz[:, :, HD + 1:HD + 2], 1.0)
        # pre-zero G buffers
        for _ in range(16):
            gz_ = p_G.tile([128, 32], BF16, tag="G", name="gzb")
            nc.gpsimd.memset(gz_[:], 0.0)

        # tail-only constants, emitted last so they never gate the loop
        ident_f32 = consts.tile([128, 128], F32, tag="ident_f32")
        make_identity(nc, ident_f32[:])
        wo_sb = consts.tile([128, DC, D], BF16, tag="wo")
        nc.sync.dma_start(wo_sb[:], wo[:, :, :])

        # ---- helpers ----
        pending_ro = []

        def emit_xt(s):
            t = p_xt.tile([128, DC, SBN], BF16, tag="xt", name="xt")
            nc.sync.dma_start(t[:, :, :], xbf[:, :, s * SBN:(s + 1) * SBN])
            return t

        def emit_qk_ec(xt_n, qkt_n, ec):
            ps = ps_big.tile([128, SBN], F32, tag="big", name="psq")
            for dc in range(DC):
                nc.tensor.matmul(
                    ps[:],
                    wqk_sb[:, dc, ec * 128:(ec + 1) * 128],
                    xt_n[:, dc, :],
                    start=(dc == 0), stop=(dc == DC - 1))
            if has_bqk:
                if ec % 2 == 0:
                    nc.vector.tensor_scalar_add(
                        qkt_n[:, ec, :], ps[:], bqk_sb[:, ec:ec + 1])
                else:
                    nc.scalar.activation(
                        qkt_n[:, ec, :], ps[:],
                        mybir.ActivationFunctionType.Identity,
                        bias=bqk_sb[:, ec:ec + 1])
            else:
                if ec % 2 == 0:
                    nc.vector.tensor_copy(qkt_n[:, ec, :], ps[:])
                else:
                    nc.scalar.copy(qkt_n[:, ec, :], ps[:])

        def emit_v(xt, b, vts):
            psv = ps_big.tile([128, SBN], F32, tag="big", name="psv")
            for dc in range(DC):
                nc.tensor.matmul(
                    psv[:],
                    xt[:, dc, b * 128:(b + 1) * 128],
                    wv_sb[:, dc, :],
                    start=(dc == 0), stop=(dc == DC - 1))
            vtx = p_vtx.tile([128, H, HD + 2], BF16, tag="vtx")
            pv = psv[:].rearrange("p (h c) -> p h c", h=H)
            if has_bv:
                nc.vector.tensor_tensor(
                    vtx[:, :, 0:HD], pv,
                    bv_full[:].rearrange("p (h c) -> p h c", h=H),
                    mybir.AluOpType.add)
            else:
                nc.vector.tensor_copy(vtx[:, :, 0:HD], pv)
            # vg[n, h] = v[n, h, :] . weff[h, :] (gate numerator seed):
            # multiply on gpsimd (has slack), reduce on DVE, scatter gpsimd
            scr = p_small.tile([128, H, HD], BF16, tag="scr")
            nc.gpsimd.tensor_tensor(
                scr[:], vtx[:, :, 0:HD],
                weff_bc[:].rearrange("p (h c) -> p h c", h=H),
                mybir.AluOpType.mult)
            vgt = p_small.tile([128, H], BF16, tag="vgt")
            with nc.allow_low_precision(reason="vg lands in bf16 vtx col"):
                nc.vector.tensor_reduce(
                    vgt[:], scr[:], mybir.AxisListType.X, mybir.AluOpType.add)
            nc.gpsimd.tensor_copy(vtx[:, :, HD], vgt[:])
            vts[b] = vtx

        def emit_scores(qkt, b, attns):
            n0 = b * BLK
            S = ps_s.tile([128, H, BLK], F32, tag="s")
            for c in range(4):
                # even head 2c -> slot c (bank A), odd 2c+1 -> slot 4+c
                # (bank B); row-tiled pairs run concurrently
                nc.tensor.matmul(
                    S[:, c, :],
                    qkt[0:64, 4 + c, n0:n0 + BLK],
                    qkt[0:64, c, n0:n0 + BLK],
                    start=True, stop=True)
                nc.tensor.matmul(
                    S[:, 4 + c, :],
                    qkt[64:128, 4 + c, n0:n0 + BLK],
                    qkt[64:128, c, n0:n0 + BLK],
                    start=True, stop=True)
            attn = p_attn.tile([128, H, BLK], BF16, tag="attn")
            nc.scalar.activation(
                attn[0:64, :, 0:64], S[0:64, :, 0:64],
                mybir.ActivationFunctionType.Exp, scale=0.125)
            nc.scalar.activation(
                attn[64:128, :, 64:128], S[64:128, :, 64:128],
                mybir.ActivationFunctionType.Exp, scale=0.125)
            attns[b] = attn

        def emit_ctx(b, attn, vtx, zp_sb, ctxns):
            psc = ps_c.tile([128, H, BLK], F32, tag="c")
            for h in range(H):
                s = (h // 2) + 4 * (h % 2)
                nc.tensor.matmul(
                    psc[:, h, 0:HD + 2],
                    attn[:, s, :],
                    vtx[:, h, :],
                    start=True, stop=True)
            rr = p_small.tile([128, H], F32, tag="rr")
            nc.vector.reciprocal(rr[:], psc[:, :, HD + 1])
            ctxn = p_ctxn.tile([128, H, HD], BF16, tag="ctxn")
            nc.vector.tensor_tensor(
                ctxn[:],
                psc[:, :, 0:HD],
                rr[:, :, None].to_broadcast((128, H, HD)),
                mybir.AluOpType.mult)
            nc.vector.tensor_tensor(
                zp_sb[:, b, :], psc[:, :, HD], rr[:],
                mybir.AluOpType.mult)
            ctxns[b] = ctxn

        def emit_gate_and_ro(sb, zp_sb, ctxns):
            gzs = p_small.tile([128, NBLK], F32, tag="gzs")
            nc.vector.tensor_reduce(
                gzs[:], zp_sb[:], mybir.AxisListType.X, mybir.AluOpType.add)
            gt = p_small.tile([128, NBLK], F32, tag="gt")
            nc.scalar.activation(
                gt[:], gzs[:], mybir.ActivationFunctionType.Tanh,
                bias=(0.5 * gb_eff) if has_gb else 0.0, scale=0.5)
            k = sb // GRP
            for b in range(NBLK):
                G = p_G.tile([128, 32], BF16, tag="G")
                c0 = 8 * (sb % GRP) + 2 * b
                nc.gpsimd.tensor_scalar(
                    G[0:64, c0:c0 + 1], gt[0:64, b:b + 1], 0.5, 0.5,
                    mybir.AluOpType.mult, mybir.AluOpType.add)
                nc.gpsimd.tensor_scalar(
                    G[64:128, c0 + 1:c0 + 2], gt[64:128, b:b + 1], 0.5, 0.5,
                    mybir.AluOpType.mult, mybir.AluOpType.add)
                if has_bo:
                    g0 = 8 * sb + 2 * b
                    nc.gpsimd.tensor_reduce(
                        s_row[0:1, g0:g0 + 1], gt[0:64, b:b + 1],
                        mybir.AxisListType.C, mybir.AluOpType.add)
                    nc.gpsimd.tensor_reduce(
                        s_row[0:1, g0 + 1:g0 + 2], gt[64:128, b:b + 1],
                        mybir.AxisListType.C, mybir.AluOpType.add)
                first = (sb % GRP == 0) and (b == 0)
                last = (sb % GRP == GRP - 1) and (b == NBLK - 1)
                cflat = ctxns[b][:].rearrange("p h c -> p (h c)")

                def _ro(G=G, cflat=cflat, k=k, first=first, last=last):
                    nc.tensor.matmul(
                        rps[32 * k:32 * k + 32, :], G[:], cflat,
                        start=first, stop=last,
                        tile_position=(0, 32 * k))
                pending_ro.append(_ro)

        # ---- prologue: QK for superblock 0 ----
        xts = {0: xt0}
        if nsb > 1:
            xts[1] = emit_xt(1)
        qkt0 = p_qkt.tile([128, H, SBN], BF16, tag="qkt", name="qkt0")
        for ec in range(8):
            emit_qk_ec(xts[0], qkt0, ec)
        qks = {0: qkt0}

        # ---- main loop ----
        for sb in range(nsb):
            if sb + 2 < nsb:
                xts[sb + 2] = emit_xt(sb + 2)
            xt = xts.pop(sb)
            qkt = qks.pop(sb)

            # QK projection groups for the NEXT superblock, interleaved
            # into this superblock's work
            if sb + 1 < nsb:
                xt_n = xts[sb + 1]
                qkt_n = p_qkt.tile([128, H, SBN], BF16, tag="qkt",
                                   name="qktn")
                qgrps = [lambda ec=ec: emit_qk_ec(xt_n, qkt_n, ec)
                         for ec in range(8)]
                qks[sb + 1] = qkt_n
            else:
                qgrps = []

            vts = [None] * NBLK
            attns = [None] * NBLK
            ctxns = [None] * NBLK
            zp_sb = p_small.tile([128, NBLK, H], F32, tag="zp_sb")
            ros = pending_ro
            pending_ro = []

            def q1(n=1):
                for _ in range(n):
                    if qgrps:
                        qgrps.pop(0)()

            def ro1(n=2):
                for _ in range(n):
                    if ros:
                        ros.pop(0)()

            # PE emission order: V groups early (vg chain latency), QK
            # groups fill exp/norm dependency gaps, readouts of the
            # previous superblock sprinkled in
            emit_v(xt, 0, vts)
            emit_scores(qkt, 0, attns)
            ro1(2)
            q1()
            emit_v(xt, 1, vts)
            ro1(2)
            q1()
            emit_scores(qkt, 1, attns)
            emit_ctx(0, attns[0], vts[0], zp_sb, ctxns)
            emit_v(xt, 2, vts)
            q1()
            emit_scores(qkt, 2, attns)
            emit_ctx(1, attns[1], vts[1], zp_sb, ctxns)
            emit_v(xt, 3, vts)
            q1()
            emit_scores(qkt, 3, attns)
            emit_ctx(2, attns[2], vts[2], zp_sb, ctxns)
            q1(2)
            emit_ctx(3, attns[3], vts[3], zp_sb, ctxns)
            q1(2)
            emit_gate_and_ro(sb, zp_sb, ctxns)

        # ---- tail: r -> r^T -> out projection ----
        while pending_ro:
            pending_ro.pop(0)()
        rsb = p_out.tile([128, D], F32, tag="rsb")
        nc.vector.tensor_copy(rsb[:], rps[:])
        ptt = ps_big.tile([128, DC, 128], F32, tag="big", name="ptt")
        for dc in range(DC):
            nc.tensor.transpose(
                ptt[:, dc, :], rsb[:, dc * 128:(dc + 1) * 128], ident_f32[:])
        rt = p_out.tile([128, DC, 128], BF16, tag="rt")
        nc.scalar.copy(rt[:], ptt[:])
        pso = ps_big.tile([128, D], F32, tag="big", name="pso")
        for dc in range(DC):
            nc.tensor.matmul(
                pso[:], rt[:, dc, :], wo_sb[:, dc, :],
                start=(dc == 0), stop=(dc == DC - 1))
        out_sb = p_out.tile([128, D], F32, tag="osb")
        if has_bo:
            # out += (sum_n gate_n) * bo : transpose s_row to [gc, 1]
            pst = ps_c.tile([128, H, BLK], F32, tag="c", name="pst")
            nc.tensor.transpose(pst[0:gc, 0, 0:1], s_row[:, :], ident_f32[:])
            s_col = p_out.tile([128, 1], F32, tag="s_col")
            nc.vector.tensor_copy(s_col[0:gc, :], pst[0:gc, 0, 0:1])
            sbo = p_out.tile([128, D], F32, tag="sbo")
            nc.vector.tensor_scalar_mul(
                sbo[:], bo_full[:], s_col[:, 0:1])
            nc.vector.tensor_tensor(
                out_sb[:], pso[:], sbo[:], mybir.AluOpType.add)
        else:
            nc.vector.tensor_copy(out_sb[:], pso[:])
        nc.sync.dma_start(out[:, :], out_sb[0:gc, :])

    import time as _time
    _t = _time.time()
    nc.compile()
    print(f"[kernel] bacc compile: {_time.time()-_t:.1f}s", flush=True)
    return nc


def kernel(x, batch, in_proj_w, in_proj_b, out_proj_w, out_proj_b,
           gate_w, gate_b):
    x = np.ascontiguousarray(np.asarray(x, dtype=np.float32))
    in_proj_w = np.asarray(in_proj_w, dtype=np.float32)
    in_proj_b = np.asarray(in_proj_b, dtype=np.float32)
    out_proj_w = np.asarray(out_proj_w, dtype=np.float32)
    out_proj_b = np.asarray(out_proj_b, dtype=np.float32)
    gate_w = np.asarray(gate_w, dtype=np.float32)
    gate_b = np.asarray(gate_b, dtype=np.float32)

    # host-side weight prep
    wqkT = in_proj_w[:2 * D].T                              # [512, 1024]
    wqk_h = np.ascontiguousarray(
        wqkT.reshape(DC, 128, 2 * D).transpose(1, 0, 2)).astype(ml_dtypes.bfloat16)
    wvT = in_proj_w[2 * D:].T                               # [512, 512]
    wv_h = np.ascontiguousarray(
        wvT.reshape(DC, 128, D).transpose(1, 0, 2)).astype(ml_dtypes.bfloat16)
    woT = out_proj_w.T                                      # [512, 512]
    wo_h = np.ascontiguousarray(
        woT.reshape(DC, 128, D).transpose(1, 0, 2)).astype(ml_dtypes.bfloat16)
    weff_h = (out_proj_w.T @ gate_w[0]).astype(np.float32).reshape(1, D)

    bqk_np = in_proj_b[:2 * D]
    bv_np = in_proj_b[2 * D:]
    gb_eff = float(gate_b[0] + out_proj_b @ gate_w[0])
    has_bqk = bool(np.any(bqk_np))
    has_bv = bool(np.any(bv_np))
    has_bo = bool(np.any(out_proj_b))
    has_gb = gb_eff != 0.0

    import time as _time
    _t = _time.time()
    nc = _build(has_bqk, has_bv, has_bo, has_gb, gb_eff=gb_eff)
    print(f"[kernel] build total: {_time.time()-_t:.1f}s", flush=True)

    in_maps = []
    for c in range(N_CORES):
        xc = x[c * ROWS:(c + 1) * ROWS].astype(ml_dtypes.bfloat16)
        xct = np.ascontiguousarray(
            xc.T.reshape(DC, 128, ROWS).transpose(1, 0, 2))
        m = {
            "xbf": xct,
            "wqk": wqk_h, "wv": wv_h, "wo": wo_h, "weff": weff_h,
        }
        if has_bqk:
            m["bqk"] = np.ascontiguousarray(
                bqk_np.reshape(8, 128).T).astype(np.float32)
        if has_bv:
            m["bv"] = bv_np.reshape(1, D).astype(np.float32)
        if has_bo:
            m["bo"] = out_proj_b.reshape(1, D).astype(np.float32)
        in_maps.append(m)

    kernel.last_nc = nc
    kernel.last_in_maps = in_maps
    kernel.last_flags = (has_bqk, has_bv, has_bo, has_gb)

    res = run_bass_kernel_spmd(
        nc, in_maps, core_ids=list(range(N_CORES)), trace=TRACE)
    if TRACE:
        kernel.last_exec_time_ns = res.exec_time_ns
        kernel.last_results = res

    return np.concatenate([r["out"] for r in res.results], axis=0)


kernel.last_exec_time_ns = None
kernel.last_results = None
kernel.last_nc = None
kernel.last_in_maps = None


# revision 20
# speedup vs baseline: 1.1331x; 1.0972x over previous
"""AttentionReadout Trainium2 kernel (8-core data-parallel over graphs).

Reference computation (per graph of 64 nodes, D=512, H=8 heads, hd=64):
    qkv = x @ in_proj_w.T + in_proj_b ; q,k,v = split(qkv)
    attn = softmax(q k^T / sqrt(hd)) v          (per head)
    attn_out = attn @ out_proj_w.T + out_proj_b
    gates = sigmoid(attn_out @ gate_w.T + gate_b)
    out[g] = sum_n attn_out[n] * gates[n]

Key algebraic restructure vs the naive chain: with weff = out_proj_w.T @
gate_w and gb_eff = gate_b + out_proj_b @ gate_w,
    gates  = sigmoid(ctx @ weff + gb_eff)           (no attn_out needed)
    out[g] = (sum_n gates_n * ctx_n) @ out_proj_w.T + (sum_n gates_n) * bo
so the out-projection runs ONCE per core on [128 graphs, D], not per node.

Layout strategy (per core: 128 graphs = 8192 nodes, superblock = 512 nodes):
  - x arrives PRE-TRANSPOSED from the host ([128, DC, rows] bf16): no
    on-device transposes, plain contiguous DMA loads only.
  - Q^T,K^T projected in [e, n] orientation one superblock AHEAD. Odd
    heads are consumed straight from partitions 64:128 via PE row-tiling
    (tile_position (64,0)) -- no realignment bounce; even/odd head score
    matmuls run concurrently in disjoint row groups.
  - scores for all 8 heads of a 128-node block go to one [128, 8, 128]
    psum (2 banks: even-head slots 0-3 bank A, odd slots 4-7 bank B so
    concurrent drains hit different banks). exp runs as TWO 512-elem
    ScalarE instructions per block (diag quadrants only; attn buffers
    pre-zeroed off-diagonal).
  - ctx natural [n, e] per head via stationary attn / moving
    [v | vg | ones] (N=66: ctx + gate numerator + rowsum in one shot)
    into one [128, 8, 128] psum; ONE reciprocal + ONE normalize + ONE
    zp instruction per block.
  - gate: zp accumulated per superblock, ONE tanh [128, 4] per sb,
    G written via 8 tiny gpsimd ops into per-block zeroed G tiles.
  - readout: per block one matmul, stationary G[128, 32], moving ctxn
    [128, 512], accumulated per 4-superblock group into an exclusive
    psum bank (col tile_position 32k) -> r[g, e] for all 128 graphs.
  - tail: r -> (PE transpose) -> r^T -> 4 matmuls vs wo -> out.
  - ~3.5us of tiny dummy matmuls at t=0 warm the PE HAM clock gate so
    real matmuls start at 2.4 GHz, hidden under the initial weight DMA.
"""

import numpy as np
import ml_dtypes

import concourse.bass as bass
import concourse.mybir as mybir
import concourse.tile as tile
from concourse import bacc
from concourse.bass_utils import run_bass_kernel_spmd
from concourse.masks import make_identity

F32 = mybir.dt.float32
BF16 = mybir.dt.bfloat16

N_CORES = 8
D = 512
H = 8
HD = 64
NPG = 64            # nodes per graph
TOTAL = 65536
ROWS = TOTAL // N_CORES      # 8192 nodes per core
GC = ROWS // NPG             # 128 graphs per core
BLK = 128                    # nodes per block (2 graphs)
SBN = 512                    # nodes per superblock (4 blocks, 8 graphs)
NSB = ROWS // SBN            # 16 superblocks
NBLK = SBN // BLK            # 4 blocks per superblock
DC = D // 128                # 4 d-chunks
GRP = 4                      # superblocks per readout group (32 graphs)
NDUM = 55                    # HAM warm-up dummy matmuls

# module-level switch used by test.py; harness default is no tracing
TRACE = False

try:
    import jax as _jax
    _jax.config.update("jax_compilation_cache_dir", "/tmp/jax_neff_cache")
    _jax.config.update("jax_persistent_cache_min_compile_time_secs", 10)
    _jax.config.update("jax_persistent_cache_min_entry_size_bytes", 0)
except Exception:
    pass


def _build(has_bqk, has_bv, has_bo, has_gb, gb_eff=0.0, rows=ROWS):
    nsb = rows // SBN
    gc = rows // NPG
    nc = bacc.Bacc(None, target_bir_lowering=False, debug=False)

    xbf = nc.dram_tensor("xbf", [128, DC, rows], BF16, kind="ExternalInput")
    wqk = nc.dram_tensor("wqk", [128, DC, 2 * D], BF16, kind="ExternalInput")
    wv = nc.dram_tensor("wv", [128, DC, D], BF16, kind="ExternalInput")
    wo = nc.dram_tensor("wo", [128, DC, D], BF16, kind="ExternalInput")
    weff = nc.dram_tensor("weff", [1, D], F32, kind="ExternalInput")
    if has_bqk:
        bqk = nc.dram_tensor("bqk", [128, 8], F32, kind="ExternalInput")
    if has_bv:
        bv = nc.dram_tensor("bv", [1, D], F32, kind="ExternalInput")
    if has_bo:
        bo = nc.dram_tensor("bo", [1, D], F32, kind="ExternalInput")
    out = nc.dram_tensor("out", [gc, D], F32, kind="ExternalOutput")

    from contextlib import ExitStack
    with tile.TileContext(nc) as tc, ExitStack() as st:
        consts = st.enter_context(tc.tile_pool(name="consts", bufs=1))
        p_xt = st.enter_context(tc.tile_pool(name="p_xt", bufs=3))
        p_qkt = st.enter_context(tc.tile_pool(name="p_qkt", bufs=2))
        p_vtx = st.enter_context(tc.tile_pool(name="p_vtx", bufs=3))
        p_attn = st.enter_context(tc.tile_pool(name="p_attn", bufs=2))
        p_ctxn = st.enter_context(tc.tile_pool(name="p_ctxn", bufs=6))
        p_small = st.enter_context(tc.tile_pool(name="p_small", bufs=3))
        p_G = st.enter_context(tc.tile_pool(name="p_G", bufs=16))
        p_out = st.enter_context(tc.tile_pool(name="p_out", bufs=1))
        ps_big = st.enter_context(tc.tile_pool(name="ps_big", bufs=3, space="PSUM"))
        ps_s = st.enter_context(tc.tile_pool(name="ps_s", bufs=1, space="PSUM"))
        ps_c = st.enter_context(tc.tile_pool(name="ps_c", bufs=1, space="PSUM"))
        ps_r = st.enter_context(tc.tile_pool(name="ps_r", bufs=1, space="PSUM"))

        # ---- HAM warm-up: matmuls on zeroed SBUF, one accumulation
        # group into a big-pool psum bank, issued before any DMA-dependent
        # work so the PE clock gate is at 8/8 when real matmuls start.
        # Also preload the ACT spline table set so the first psum->sbuf
        # copies don't eat the ~2.7us ACT_TABLE_LOAD.
        zmov = consts.tile([64, 64], BF16, tag="zmov")
        nc.gpsimd.memset(zmov[:], 0.0)
        actwarm = consts.tile([1, 1], F32, tag="actwarm")
        nc.scalar.activation(
            actwarm[:], zmov[0:1, 0:1],
            mybir.ActivationFunctionType.Exp, scale=1.0)
        dumps = ps_big.tile([128, SBN], F32, tag="big", name="dumps")
        for i in range(NDUM):
            nc.tensor.matmul(
                dumps[0:64, 0:64], zmov[:], zmov[:],
                start=(i == 0), stop=(i == NDUM - 1))

        # ---- weights / constants; order matters for startup overlap ----
        xt0 = p_xt.tile([128, DC, SBN], BF16, tag="xt", name="xt0")
        nc.sync.dma_start(xt0[:, :, :], xbf[:, :, 0:SBN])
        wv_sb = consts.tile([128, DC, D], BF16, tag="wv")
        nc.sync.dma_start(wv_sb[:], wv[:, :, :])
        wqk_sb = consts.tile([128, DC, 2 * D], BF16, tag="wqk")
        for q in range(4):
            nc.sync.dma_start(
                wqk_sb[:, :, q * D // 2:(q + 1) * D // 2],
                wqk[:, :, q * D // 2:(q + 1) * D // 2])

        weff_row = consts.tile([1, D], F32, tag="weff_row")
        nc.sync.dma_start(weff_row[:], weff[:, :])
        weff_f32 = consts.tile([128, D], F32, tag="weff_f32")
        nc.gpsimd.partition_broadcast(weff_f32[:], weff_row[:])
        weff_bc = consts.tile([128, D], BF16, tag="weff_bc")
        nc.vector.tensor_copy(weff_bc[:], weff_f32[:])

        if has_bqk:
            bqk_sb = consts.tile([128, 8], F32, tag="bqk")
            nc.sync.dma_start(bqk_sb[:], bqk[:, :])
        if has_bv:
            bv_row = consts.tile([1, D], F32, tag="bv_row")
            nc.sync.dma_start(bv_row[:], bv[:, :])
            bv_full = consts.tile([128, D], F32, tag="bv_full")
            nc.gpsimd.partition_broadcast(bv_full[:], bv_row[:])
        if has_bo:
            bo_row = consts.tile([1, D], F32, tag="bo_row")
            nc.sync.dma_start(bo_row[:], bo[:, :])
            bo_full = consts.tile([128, D], F32, tag="bo_full")
            nc.gpsimd.partition_broadcast(bo_full[:], bo_row[:])
            s_row = consts.tile([1, gc], F32, tag="s_row")

        # readout accumulator: one psum bank, exclusively owned
        rps = ps_r.tile([128, D], F32, tag="r")

        # persistent double-buffer rings (explicitly indexed; their
        # initialized regions -- attn off-diagonal zeros, vtx ones
        # column, G zeros -- persist across reuses)
        attn_ring = []
        for i in range(2):
            az = p_attn.tile([128, H, BLK], BF16, tag=f"attn{i}",
                             name=f"az{i}")
            nc.gpsimd.memset(az[0:64, :, 64:128], 0.0)
            nc.gpsimd.memset(az[64:128, :, 0:64], 0.0)
            attn_ring.append(az)
        # vtx layout per block/head: [v 0:64 | vg 64 | ones 65]
        vtx_ring = []
        for i in range(2):
            vz = p_vtx.tile([128, NBLK, H, HD + 2], BF16, tag=f"vtx{i}",
                            name=f"vz{i}")
            nc.vector.memset(vz[:, :, :, HD + 1:HD + 2], 1.0)
            vtx_ring.append(vz)
        # G tiles: one per (sb%GRP, block); only that block's 2 columns
        # are ever written, the rest stay zero forever
        g_ring = []
        for i in range(GRP * NBLK):
            gz_ = p_G.tile([128, 32], BF16, tag=f"G{i}", name=f"gzb{i}")
            nc.gpsimd.memset(gz_[:], 0.0)
            g_ring.append(gz_)

        # tail-only constants, emitted last so they never gate the loop
        ident_f32 = consts.tile([128, 128], F32, tag="ident_f32")
        make_identity(nc, ident_f32[:])
        wo_sb = consts.tile([128, DC, D], BF16, tag="wo")
        nc.sync.dma_start(wo_sb[:], wo[:, :, :])

        # ---- helpers ----
        pending_ro = []

        def emit_xt(s):
            t = p_xt.tile([128, DC, SBN], BF16, tag="xt", name="xt")
            nc.sync.dma_start(t[:, :, :], xbf[:, :, s * SBN:(s + 1) * SBN])
            return t

        def emit_qk_ec(xt_n, qkt_n, ec):
            ps = ps_big.tile([128, SBN], F32, tag="big", name="psq")
            for dc in range(DC):
                nc.tensor.matmul(
                    ps[:],
                    wqk_sb[:, dc, ec * 128:(ec + 1) * 128],
                    xt_n[:, dc, :],
                    start=(dc == 0), stop=(dc == DC - 1))
            if has_bqk:
                if ec % 2 == 0:
                    nc.vector.tensor_scalar_add(
                        qkt_n[:, ec, :], ps[:], bqk_sb[:, ec:ec + 1])
                else:
                    nc.scalar.activation(
                        qkt_n[:, ec, :], ps[:],
                        mybir.ActivationFunctionType.Identity,
                        bias=bqk_sb[:, ec:ec + 1])
            else:
                if ec % 2 == 0:
                    nc.vector.tensor_copy(qkt_n[:, ec, :], ps[:])
                else:
                    nc.scalar.copy(qkt_n[:, ec, :], ps[:])

        def emit_v(xt, b, vtx, act_copy=False):
            psv = ps_big.tile([128, SBN], F32, tag="big", name="psv")
            for dc in range(DC):
                nc.tensor.matmul(
                    psv[:],
                    xt[:, dc, b * 128:(b + 1) * 128],
                    wv_sb[:, dc, :],
                    start=(dc == 0), stop=(dc == DC - 1))
            pv = psv[:].rearrange("p (h c) -> p h c", h=H)
            if has_bv:
                nc.vector.tensor_tensor(
                    vtx[:, b, :, 0:HD], pv,
                    bv_full[:].rearrange("p (h c) -> p h c", h=H),
                    mybir.AluOpType.add)
            elif act_copy:
                nc.scalar.copy(vtx[:, b, :, 0:HD], pv)
            else:
                nc.vector.tensor_copy(vtx[:, b, :, 0:HD], pv)

        def emit_vg(vtx, b0):
            # vg[n, h] = v[n, h, :] . weff[h, :] (gate numerator seed),
            # batched over two blocks: bf16 DVE multiply + reduce, gpsimd
            # scatter into the vtx vg columns
            scr = p_small.tile([128, 2, H, HD], BF16, tag="scr")
            nc.vector.tensor_tensor(
                scr[:], vtx[:, b0:b0 + 2, :, 0:HD],
                weff_bc[:].rearrange("p (h c) -> p h c", h=H)[
                    :, None, :, :].to_broadcast((128, 2, H, HD)),
                mybir.AluOpType.mult)
            vgt = p_small.tile([128, 2, H], BF16, tag="vgt")
            with nc.allow_low_precision(reason="vg lands in bf16 vtx col"):
                nc.vector.tensor_reduce(
                    vgt[:], scr[:], mybir.AxisListType.X, mybir.AluOpType.add)
            nc.gpsimd.tensor_copy(vtx[:, b0:b0 + 2, :, HD], vgt[:])

        def emit_scores(qkt, b, attns):
            n0 = b * BLK
            S = ps_s.tile([128, H, BLK], F32, tag="s")
            for c in range(4):
                # even head 2c -> slot c (bank A), odd 2c+1 -> slot 4+c
                # (bank B); row-tiled pairs run concurrently
                nc.tensor.matmul(
                    S[:, c, :],
                    qkt[0:64, 4 + c, n0:n0 + BLK],
                    qkt[0:64, c, n0:n0 + BLK],
                    start=True, stop=True)
                nc.tensor.matmul(
                    S[:, 4 + c, :],
                    qkt[64:128, 4 + c, n0:n0 + BLK],
                    qkt[64:128, c, n0:n0 + BLK],
                    start=True, stop=True)
            attn = attn_ring[b % 2]
            nc.scalar.activation(
                attn[0:64, :, 0:64], S[0:64, :, 0:64],
                mybir.ActivationFunctionType.Exp, scale=0.125)
            nc.scalar.activation(
                attn[64:128, :, 64:128], S[64:128, :, 64:128],
                mybir.ActivationFunctionType.Exp, scale=0.125)
            attns[b] = attn

        def emit_ctx(b, attn, vtx, zp_sb, ctxns):
            psc = ps_c.tile([128, H, BLK], F32, tag="c")
            for h in range(H):
                s = (h // 2) + 4 * (h % 2)
                nc.tensor.matmul(
                    psc[:, h, 0:HD + 2],
                    attn[:, s, :],
                    vtx[:, b, h, :],
                    start=True, stop=True)
            rr = p_small.tile([128, H], F32, tag="rr")
            nc.vector.reciprocal(rr[:], psc[:, :, HD + 1])
            ctxn = p_ctxn.tile([128, H, HD], BF16, tag="ctxn")
            nc.vector.tensor_tensor(
                ctxn[:],
                psc[:, :, 0:HD],
                rr[:, :, None].to_broadcast((128, H, HD)),
                mybir.AluOpType.mult)
            nc.vector.tensor_tensor(
                zp_sb[:, b, :], psc[:, :, HD], rr[:],
                mybir.AluOpType.mult)
            ctxns[b] = ctxn

        def emit_gate_and_ro(sb, zp_sb, ctxns):
            gzs = p_small.tile([128, NBLK], F32, tag="gzs")
            nc.vector.tensor_reduce(
                gzs[:], zp_sb[:], mybir.AxisListType.X, mybir.AluOpType.add)
            gt = p_small.tile([128, NBLK], F32, tag="gt")
            nc.scalar.activation(
                gt[:], gzs[:], mybir.ActivationFunctionType.Tanh,
                bias=(0.5 * gb_eff) if has_gb else 0.0, scale=0.5)
            k = sb // GRP
            for b in range(NBLK):
                G = g_ring[(sb % GRP) * NBLK + b]
                c0 = 8 * (sb % GRP) + 2 * b
                nc.gpsimd.tensor_scalar(
                    G[0:64, c0:c0 + 1], gt[0:64, b:b + 1], 0.5, 0.5,
                    mybir.AluOpType.mult, mybir.AluOpType.add)
                nc.gpsimd.tensor_scalar(
                    G[64:128, c0 + 1:c0 + 2], gt[64:128, b:b + 1], 0.5, 0.5,
                    mybir.AluOpType.mult, mybir.AluOpType.add)
                if has_bo:
                    g0 = 8 * sb + 2 * b
                    nc.gpsimd.tensor_reduce(
                        s_row[0:1, g0:g0 + 1], gt[0:64, b:b + 1],
                        mybir.AxisListType.C, mybir.AluOpType.add)
                    nc.gpsimd.tensor_reduce(
                        s_row[0:1, g0 + 1:g0 + 2], gt[64:128, b:b + 1],
                        mybir.AxisListType.C, mybir.AluOpType.add)
                first = (sb % GRP == 0) and (b == 0)
                last = (sb % GRP == GRP - 1) and (b == NBLK - 1)
                cflat = ctxns[b][:].rearrange("p h c -> p (h c)")

                def _ro(G=G, cflat=cflat, k=k, first=first, last=last):
                    nc.tensor.matmul(
                        rps[32 * k:32 * k + 32, :], G[:], cflat,
                        start=first, stop=last,
                        tile_position=(0, 32 * k))
                pending_ro.append(_ro)

        # ---- prologue: V for superblock 0, then QK for superblock 0 ----
        xts = {0: xt0}
        if nsb > 1:
            xts[1] = emit_xt(1)
        vtx0 = vtx_ring[0]
        for b in range(NBLK):
            emit_v(xts[0], b, vtx0, act_copy=(b % 2 == 1))
        emit_vg(vtx0, 0)
        emit_vg(vtx0, 2)
        qkt0 = p_qkt.tile([128, H, SBN], BF16, tag="qkt", name="qkt0")
        for ec in range(8):
            emit_qk_ec(xts[0], qkt0, ec)
        qks = {0: qkt0}
        vtxs = {0: vtx0}

        # ---- main loop ----
        for sb in range(nsb):
            if sb + 2 < nsb:
                xts[sb + 2] = emit_xt(sb + 2)
            xt = xts.pop(sb)
            qkt = qks.pop(sb)
            vtx = vtxs.pop(sb)

            # QK projection groups for the NEXT superblock, interleaved
            # into this superblock's work; V groups for the next
            # superblock likewise (their vtx is consumed next sb)
            if sb + 1 < nsb:
                xt_n = xts[sb + 1]
                qkt_n = p_qkt.tile([128, H, SBN], BF16, tag="qkt",
                                   name="qktn")
                qgrps = [lambda ec=ec: emit_qk_ec(xt_n, qkt_n, ec)
                         for ec in range(8)]
                qks[sb + 1] = qkt_n
                vtx_n = vtx_ring[(sb + 1) % 2]
                vgrps = [lambda b=b: emit_v(xt_n, b, vtx_n,
                                            act_copy=(b % 2 == 1))
                         for b in range(NBLK)]
                vtxs[sb + 1] = vtx_n
            else:
                qgrps = []
                vgrps = []
                vtx_n = None

            attns = [None] * NBLK
            ctxns = [None] * NBLK
            zp_sb = p_small.tile([128, NBLK, H], F32, tag="zp_sb")
            ros = pending_ro
            pending_ro = []

            def q1(n=1):
                for _ in range(n):
                    if qgrps:
                        qgrps.pop(0)()

            def v1(n=1):
                for _ in range(n):
                    if vgrps:
                        vgrps.pop(0)()

            def ro1(n=2):
                for _ in range(n):
                    if ros:
                        ros.pop(0)()

            # PE emission order: the next superblock's V/QK groups fill
            # this superblock's exp/norm dependency gaps; vg chains for
            # the next sb are emitted right after the pair of V copies
            # they need, readouts of the previous superblock early
            emit_scores(qkt, 0, attns)
            ro1(4)
            v1()
            q1()
            emit_scores(qkt, 1, attns)
            emit_ctx(0, attns[0], vtx, zp_sb, ctxns)
            v1()
            if vtx_n is not None:
                emit_vg(vtx_n, 0)
            q1()
            emit_scores(qkt, 2, attns)
            emit_ctx(1, attns[1], vtx, zp_sb, ctxns)
            v1()
            q1()
            emit_scores(qkt, 3, attns)
            emit_ctx(2, attns[2], vtx, zp_sb, ctxns)
            v1()
            if vtx_n is not None:
                emit_vg(vtx_n, 2)
            q1()
            emit_ctx(3, attns[3], vtx, zp_sb, ctxns)
            q1(4)
            emit_gate_and_ro(sb, zp_sb, ctxns)

        # ---- tail: r -> r^T -> out projection ----
        while pending_ro:
            pending_ro.pop(0)()
        rsb = p_out.tile([128, D], F32, tag="rsb")
        nc.vector.tensor_copy(rsb[:], rps[:])
        ptt = ps_big.tile([128, DC, 128], F32, tag="big", name="ptt")
        for dc in range(DC):
            nc.tensor.transpose(
                ptt[:, dc, :], rsb[:, dc * 128:(dc + 1) * 128], ident_f32[:])
        rt = p_out.tile([128, DC, 128], BF16, tag="rt")
        nc.scalar.copy(rt[:], ptt[:])
        pso = ps_big.tile([128, D], F32, tag="big", name="pso")
        for dc in range(DC):
            nc.tensor.matmul(
                pso[:], rt[:, dc, :], wo_sb[:, dc, :],
                start=(dc == 0), stop=(dc == DC - 1))
        out_sb = p_out.tile([128, D], F32, tag="osb")
        if has_bo:
            # out += (sum_n gate_n) * bo : transpose s_row to [gc, 1]
            pst = ps_c.tile([128, H, BLK], F32, tag="c", name="pst")
            nc.tensor.transpose(pst[0:gc, 0, 0:1], s_row[:, :], ident_f32[:])
            s_col = p_out.tile([128, 1], F32, tag="s_col")
            nc.vector.tensor_copy(s_col[0:gc, :], pst[0:gc, 0, 0:1])
            sbo = p_out.tile([128, D], F32, tag="sbo")
            nc.vector.tensor_scalar_mul(
                sbo[:], bo_full[:], s_col[:, 0:1])
            nc.vector.tensor_tensor(
                out_sb[:], pso[:], sbo[:], mybir.AluOpType.add)
        else:
            nc.vector.tensor_copy(out_sb[:], pso[:])
        nc.sync.dma_start(out[:, :], out_sb[0:gc, :])

    import time as _time
    _t = _time.time()
    nc.compile()
    print(f"[kernel] bacc compile: {_time.time()-_t:.1f}s", flush=True)
    return nc


def kernel(x, batch, in_proj_w, in_proj_b, out_proj_w, out_proj_b,
           gate_w, gate_b):
    x = np.ascontiguousarray(np.asarray(x, dtype=np.float32))
    in_proj_w = np.asarray(in_proj_w, dtype=np.float32)
    in_proj_b = np.asarray(in_proj_b, dtype=np.float32)
    out_proj_w = np.asarray(out_proj_w, dtype=np.float32)
    out_proj_b = np.asarray(out_proj_b, dtype=np.float32)
    gate_w = np.asarray(gate_w, dtype=np.float32)
    gate_b = np.asarray(gate_b, dtype=np.float32)

    # host-side weight prep
    wqkT = in_proj_w[:2 * D].T                              # [512, 1024]
    wqk_h = np.ascontiguousarray(
        wqkT.reshape(DC, 128, 2 * D).transpose(1, 0, 2)).astype(ml_dtypes.bfloat16)
    wvT = in_proj_w[2 * D:].T                               # [512, 512]
    wv_h = np.ascontiguousarray(
        wvT.reshape(DC, 128, D).transpose(1, 0, 2)).astype(ml_dtypes.bfloat16)
    woT = out_proj_w.T                                      # [512, 512]
    wo_h = np.ascontiguousarray(
        woT.reshape(DC, 128, D).transpose(1, 0, 2)).astype(ml_dtypes.bfloat16)
    weff_h = (out_proj_w.T @ gate_w[0]).astype(np.float32).reshape(1, D)

    bqk_np = in_proj_b[:2 * D]
    bv_np = in_proj_b[2 * D:]
    gb_eff = float(gate_b[0] + out_proj_b @ gate_w[0])
    has_bqk = bool(np.any(bqk_np))
    has_bv = bool(np.any(bv_np))
    has_bo = bool(np.any(out_proj_b))
    has_gb = gb_eff != 0.0

    import time as _time
    _t = _time.time()
    nc = _build(has_bqk, has_bv, has_bo, has_gb, gb_eff=gb_eff)
    print(f"[kernel] build total: {_time.time()-_t:.1f}s", flush=True)

    in_maps = []
    for c in range(N_CORES):
        xc = x[c * ROWS:(c + 1) * ROWS].astype(ml_dtypes.bfloat16)
        xct = np.ascontiguousarray(
            xc.T.reshape(DC, 128, ROWS).transpose(1, 0, 2))
        m = {
            "xbf": xct,
            "wqk": wqk_h, "wv": wv_h, "wo": wo_h, "weff": weff_h,
        }
        if has_bqk:
            m["bqk"] = np.ascontiguousarray(
                bqk_np.reshape(8, 128).T).astype(np.float32)
        if has_bv:
            m["bv"] = bv_np.reshape(1, D).astype(np.float32)
        if has_bo:
            m["bo"] = out_proj_b.reshape(1, D).astype(np.float32)
        in_maps.append(m)

    kernel.last_nc = nc
    kernel.last_in_maps = in_maps
    kernel.last_flags = (has_bqk, has_bv, has_bo, has_gb)

    res = run_bass_kernel_spmd(
        nc, in_maps, core_ids=list(range(N_CORES)), trace=TRACE)
    if TRACE:
        kernel.last_exec_time_ns = res.exec_time_ns
        kernel.last_results = res

    return np.concatenate([r["out"] for r in res.results], axis=0)


kernel.last_exec_time_ns = None
kernel.last_results = None
kernel.last_nc = None
kernel.last_in_maps = None
